# revision 14
# baseline (speedup 1.0000x reference)
"""CGCNN (nn_CGCNNModel) on 8 trn2 NeuronCores via Bass/Tile SPMD.

V4 design (h-table + batched gathers + SBUF-resident state):
  - edges sorted by dst; nodes split into 8 contiguous ranges of 12500; each
    core's nodes greedy-packed into G groups (<=128 nodes, <=KT*128=1024 edge
    slots).
  - The AllGathered table holds h itself (bf16, 256B rows) instead of the
    pre-transformed Bf rows (512B) -> collective halved. Per tile the gathered
    h_src block is PE-transposed and multiplied by Wj.
  - ONE batched indirect gather per group (offsets [128, KT]) for h_src and a
    second for Af[dst] (dst-side linear term, gathered from a per-group DRAM
    staging row-block) -> SWDGE fixed overhead amortized 8x, and no per-tile
    one-hot transpose for the Af scatter.
  - h (f32), hT (bf16) and agg (bf16) live in SBUF for the whole kernel;
    BN apply is fused into the next layer's phase A / the readout. No h/agg/Af
    DRAM round trips.
  - activation tables pinned so the per-tile Exp/Ln pair never swaps tables.
  - gate = 1/(1+exp(-xg)) via wide Exp (gate weights pre-negated), then
    den=TS-add, msg = corr/den via DVE divide. All elementwise in bf16.
  - BN stats via ones-matmuls over bf16 agg + AllReduce[256]; readout:
    windowed one-hot pooling matmuls + AllReduce + tiny MLP.
"""
import sys
import numpy as np

sys.path.insert(0, "/opt/trn_rl_repo")

import ml_dtypes

import concourse.bass as bass
import concourse.mybir as mybir
import concourse.tile as tile
from concourse import bacc
from concourse.bass_utils import run_bass_kernel_spmd
from concourse.masks import make_identity

# problem constants (hardcoded per contract)
N_NODES = 100000
N_EDGES = 800000
N_GRAPHS = 1000
F_NODE = 92
F_EDGE = 80
H = 128
N_CONV = 3
BN_EPS = 1e-5

NCORES = 8
NLOC = N_NODES // NCORES      # 12500 nodes per core
KT = 8                        # tiles per group
GSLOTS = KT * 128             # 1024 edge slots per group
GB_CAP = 1024                 # graph buffer rows
OOB = 1 << 30

P = 128
f32 = mybir.dt.float32
bf16 = mybir.dt.bfloat16
i32 = mybir.dt.int32
AF = mybir.ActivationFunctionType
ALU = mybir.AluOpType

_CACHE = {}


def _bf(x):
    return np.ascontiguousarray(x).astype(ml_dtypes.bfloat16)


def pack_host(x, edge_attr, edge_index, batch, params):
    src = np.asarray(edge_index[0]).astype(np.int64)
    dst = np.asarray(edge_index[1]).astype(np.int64)
    ea = np.asarray(edge_attr, dtype=np.float32)
    batch = np.asarray(batch).astype(np.int64)
    x = np.asarray(x, dtype=np.float32)

    order = np.argsort(dst, kind="stable")
    dst_s, src_s, ea_s = dst[order], src[order], ea[order]
    deg = np.bincount(dst_s, minlength=N_NODES)
    estart = np.zeros(N_NODES + 1, dtype=np.int64)
    np.cumsum(deg, out=estart[1:])

    core_groups = []
    for c in range(NCORES):
        nlo, nhi = c * NLOC, (c + 1) * NLOC
        groups = []
        n = nlo
        while n < nhi:
            cnt = 0
            edges = 0
            while (n + cnt < nhi and cnt < 128
                   and edges + deg[n + cnt] <= GSLOTS):
                edges += int(deg[n + cnt])
                cnt += 1
            assert cnt > 0
            groups.append((n, cnt, int(estart[n]), edges))
            n += cnt
        core_groups.append(groups)
    G = max(len(g) for g in core_groups)
    ES = G * GSLOTS
    T = G * KT
    GR = G * P

    # node -> global row id in the AllGathered h table
    grow = np.zeros(N_NODES, dtype=np.int64)
    for c in range(NCORES):
        for g, (n0, cnt, e0, ecnt) in enumerate(core_groups[c]):
            grow[n0:n0 + cnt] = c * GR + g * P + np.arange(cnt)

    in_maps = []
    for c in range(NCORES):
        nlo = c * NLOC
        groups = core_groups[c]
        src_slot = np.zeros(ES, dtype=np.int64)        # pad -> row 0 (finite)
        dloc_slot = np.full(ES, 128, dtype=np.float32)  # pad -> no one-hot col
        ea_slot = np.zeros((ES, F_EDGE), dtype=np.float32)
        xg = np.zeros((F_NODE + 1, GR), dtype=np.float32)
        for g, (n0, cnt, e0, ecnt) in enumerate(groups):
            b = g * GSLOTS
            src_slot[b:b + ecnt] = grow[src_s[e0:e0 + ecnt]]
            dl = (dst_s[e0:e0 + ecnt] - n0)
            dloc_slot[b:b + ecnt] = dl.astype(np.float32)
            ea_slot[b:b + ecnt] = ea_s[e0:e0 + ecnt]
            xg[:F_NODE, g * P:g * P + cnt] = x[n0:n0 + cnt].T
            xg[F_NODE, g * P:g * P + cnt] = 1.0

        eaT = np.concatenate([ea_slot.T, np.ones((1, ES), np.float32)], axis=0)
        srcT = src_slot.reshape(T, P).T.astype(np.int32).copy()
        dstc = dloc_slot.reshape(T, P).T.copy()

        # pooling (group-padded rows; pad rows get zero weights)
        bl = batch[nlo:nlo + NLOC]
        g_lo = int(bl[0])
        span = int(bl[-1]) - g_lo + 1
        assert span <= 256, f"graph span {span} exceeds 2 windows"
        poolw = np.zeros((P, G * 256), dtype=np.float32)
        for g, (n0, cnt, e0, ecnt) in enumerate(groups):
            gb = batch[n0:n0 + cnt] - g_lo
            pr = np.arange(cnt)
            w = (gb // 128).astype(np.int64)
            q = (gb % 128).astype(np.int64)
            poolw[pr, g * 256 + w * 128 + q] = 1.0
        pids = np.zeros((P, 2), dtype=np.int32)
        for w in range(2):
            r = g_lo + w * 128 + np.arange(P)
            pids[:, w] = np.where(r < GB_CAP, r, OOB).astype(np.int32)

        cnts = np.bincount(batch, minlength=GB_CAP).astype(np.float32)
        invc = (1.0 / np.maximum(cnts[:GB_CAP], 1.0)).reshape(8, P).T.copy()

        lw_f = np.asarray(params["lin_f_W"], np.float32)
        lw_s = np.asarray(params["lin_s_W"], np.float32)
        lb_f = np.asarray(params["lin_f_b"], np.float32)
        lb_s = np.asarray(params["lin_s_b"], np.float32)
        # gate halves NEGATED so a single wide Exp computes [exp(-xg)|exp(xc)]
        wi_all = np.concatenate(
            [np.concatenate([-lw_f[l, :128], lw_s[l, :128]], axis=1)
             for l in range(N_CONV)], axis=1)
        wj_all = np.concatenate(
            [np.concatenate([-lw_f[l, 128:256], lw_s[l, 128:256]], axis=1)
             for l in range(N_CONV)], axis=1)
        wfs_all = np.concatenate(
            [np.concatenate(
                [np.concatenate([-lw_f[l, 256:], lw_s[l, 256:]], axis=1),
                 np.concatenate([-lb_f[l], lb_s[l]])[None, :]], axis=0)
             for l in range(N_CONV)], axis=1)          # [81, 768]
        wemb = np.concatenate([np.asarray(params["emb_W"], np.float32),
                               np.asarray(params["emb_b"], np.float32)[None, :]], axis=0)

        m = {
            "xg": _bf(xg),
            "eaT": _bf(eaT),
            "srcT": srcT,
            "dstc": dstc.astype(np.float32),
            "wi": _bf(wi_all),
            "wj": _bf(wj_all),
            "wfs": _bf(wfs_all),
            "wemb": _bf(wemb),
            "bng": np.asarray(params["bn_gamma"], np.float32).reshape(N_CONV, H),
            "bnb": np.asarray(params["bn_beta"], np.float32).reshape(N_CONV, H),
            "poolw": _bf(poolw),
            "pids": pids,
            "invc": invc,
            "fcw": np.asarray(params["fc_W"], np.float32),
            "fcb": np.asarray(params["fc_b"], np.float32).reshape(1, H),
            "outw": np.asarray(params["out_W"], np.float32).reshape(H)[None, :],
            "outb": np.full((P, 1), float(np.asarray(params["out_b"]).reshape(-1)[0]), np.float32),
        }
        in_maps.append(m)
    return in_maps, G


def _pin_act_tables(nc):
    """Restrict the activation-table placement pass to table sets that serve
    {Exp, Ln} together plus one for Sqrt, so the per-tile Exp/Ln pair never
    swaps tables (1283ns per swap otherwise). act_func_set_id is positional,
    so keep the full list in order and blank out unwanted sets."""
    import types
    import bass_rust as _bass_rust
    from concourse.hw_specs import get_activation_tables

    def pinned(self):
        has_act = any(isinstance(i, mybir.InstActivation)
                      for b in self.main_func.blocks for i in b.instructions)
        if not has_act:
            return
        tables = get_activation_tables(self.m.arch)
        AFT = mybir.ActivationFunctionType
        keep = [(n, s if ((AFT.Exp in s and AFT.Ln in s) or AFT.Sqrt in s)
                 else set())
                for n, s in tables.items()]
        assert any(s for _, s in keep), f"no usable act tables in {list(tables)}"
        _bass_rust.insert_act_table_loads(self, keep)

    nc.insert_act_table_loads = types.MethodType(pinned, nc)


def build_program(G, reps=1):
    ES = G * GSLOTS
    T = G * KT
    GR = G * P
    nc = bacc.Bacc("TRN2", target_bir_lowering=False, debug=False, num_devices=NCORES)
    _pin_act_tables(nc)
    CORES = list(range(NCORES))

    xg_d = nc.dram_tensor("xg", [F_NODE + 1, GR], bf16, kind="ExternalInput")
    eaT_d = nc.dram_tensor("eaT", [F_EDGE + 1, ES], bf16, kind="ExternalInput")
    srcT_d = nc.dram_tensor("srcT", [P, T], i32, kind="ExternalInput")
    dstc_d = nc.dram_tensor("dstc", [P, T], f32, kind="ExternalInput")
    wi_d = nc.dram_tensor("wi", [H, 2 * H * N_CONV], bf16, kind="ExternalInput")
    wj_d = nc.dram_tensor("wj", [H, 2 * H * N_CONV], bf16, kind="ExternalInput")
    wfs_d = nc.dram_tensor("wfs", [F_EDGE + 1, 2 * H * N_CONV], bf16, kind="ExternalInput")
    wemb_d = nc.dram_tensor("wemb", [F_NODE + 1, H], bf16, kind="ExternalInput")
    bng_d = nc.dram_tensor("bng", [N_CONV, H], f32, kind="ExternalInput")
    bnb_d = nc.dram_tensor("bnb", [N_CONV, H], f32, kind="ExternalInput")
    poolw_d = nc.dram_tensor("poolw", [P, G * 256], bf16, kind="ExternalInput")
    pids_d = nc.dram_tensor("pids", [P, 2], i32, kind="ExternalInput")
    invc_d = nc.dram_tensor("invc", [P, GB_CAP // P], f32, kind="ExternalInput")
    fcw_d = nc.dram_tensor("fcw", [H, H], f32, kind="ExternalInput")
    fcb_d = nc.dram_tensor("fcb", [1, H], f32, kind="ExternalInput")
    outw_d = nc.dram_tensor("outw", [1, H], f32, kind="ExternalInput")
    outb_d = nc.dram_tensor("outb", [P, 1], f32, kind="ExternalInput")
    out_d = nc.dram_tensor("out", [GB_CAP, 1], f32, kind="ExternalOutput")

    hin = [nc.dram_tensor(f"hin{i}", [GR, H], bf16) for i in range(2)]
    hag = [nc.dram_tensor(f"hag{i}", [NCORES * GR, H], bf16, addr_space="Shared")
           for i in range(2)]
    st_in = [nc.dram_tensor(f"st_in{l}", [1, 2 * H], f32) for l in range(N_CONV)]
    st_out = [nc.dram_tensor(f"st_out{l}", [1, 2 * H], f32, addr_space="Shared")
              for l in range(N_CONV)]
    gbuf = nc.dram_tensor("gbuf", [GB_CAP, H], f32)
    gsum = nc.dram_tensor("gsum", [GB_CAP, H], f32, addr_space="Shared")

    with tile.TileContext(nc) as tc:
        with (
            tc.tile_pool(name="cst", bufs=1) as cst,
            tc.tile_pool(name="sb", bufs=3) as sb,
            tc.tile_pool(name="sg", bufs=3) as sg,
            tc.tile_pool(name="sc", bufs=1) as sc,
            tc.tile_pool(name="bnc", bufs=2) as bnc,
            tc.tile_pool(name="psp", bufs=2, space="PSUM") as psp,
            tc.tile_pool(name="pst", bufs=2, space="PSUM") as pst,
            tc.tile_pool(name="psa", bufs=2, space="PSUM") as psa,
            tc.tile_pool(name="psaf", bufs=1, space="PSUM") as psaf,
            tc.tile_pool(name="psst", bufs=1, space="PSUM") as psst,
        ):
            id_bf = cst.tile([P, P], bf16)
            make_identity(nc, id_bf[:])
            id_f = cst.tile([P, P], f32)
            make_identity(nc, id_f[:])
            iota_i = cst.tile([P, P], i32)
            nc.gpsimd.iota(iota_i[:], pattern=[[1, P]], base=0, channel_multiplier=0)
            iota_row = cst.tile([P, P], bf16)
            nc.vector.tensor_copy(out=iota_row[:], in_=iota_i[:])
            ones_col_bf = cst.tile([P, 1], bf16)
            nc.vector.memset(ones_col_bf[:], 1.0)
            ones_row = cst.tile([1, P], f32)
            nc.vector.memset(ones_row[:], 1.0)

            srcT_t = cst.tile([P, T], i32)
            nc.sync.dma_start(out=srcT_t[:], in_=srcT_d[:])
            dstc_t = cst.tile([P, T], f32)
            nc.sync.dma_start(out=dstc_t[:], in_=dstc_d[:])
            wi_t = cst.tile([H, 2 * H * N_CONV], bf16)
            nc.sync.dma_start(out=wi_t[:], in_=wi_d[:])
            wj_t = cst.tile([H, 2 * H * N_CONV], bf16)
            nc.sync.dma_start(out=wj_t[:], in_=wj_d[:])
            wfs_t = cst.tile([F_EDGE + 1, 2 * H * N_CONV], bf16)
            nc.sync.dma_start(out=wfs_t[:], in_=wfs_d[:])
            wemb_t = cst.tile([F_NODE + 1, H], bf16)
            nc.sync.dma_start(out=wemb_t[:], in_=wemb_d[:])
            pids_t = cst.tile([P, 2], i32)
            nc.sync.dma_start(out=pids_t[:], in_=pids_d[:])
            invc_t = cst.tile([P, GB_CAP // P], f32)
            nc.sync.dma_start(out=invc_t[:], in_=invc_d[:])
            fcw_t = cst.tile([H, H], f32)
            nc.sync.dma_start(out=fcw_t[:], in_=fcw_d[:])
            fcb_t = cst.tile([1, H], f32)
            nc.sync.dma_start(out=fcb_t[:], in_=fcb_d[:])
            outw_t = cst.tile([1, H], f32)
            nc.sync.dma_start(out=outw_t[:], in_=outw_d[:])
            outb_t = cst.tile([P, 1], f32)
            nc.sync.dma_start(out=outb_t[:], in_=outb_d[:])

            # persistent per-core state
            h_state = cst.tile([P, G * H], f32)
            hT_buf = cst.tile([P, G * H], bf16)
            agg_res = cst.tile([P, G * 2 * H], bf16)

            for _rep in range(reps):
                # ---- embedding: h_state[g] = xg_g.T @ wemb ----
                for g in range(G):
                    xgg = sb.tile([F_NODE + 1, P], bf16, tag="xgg")
                    nc.sync.dma_start(out=xgg[:], in_=xg_d[:, g * P:(g + 1) * P])
                    ps = psp.tile([P, H], f32, space="PSUM", tag="pp")
                    nc.tensor.matmul(out=ps[:], lhsT=xgg[:], rhs=wemb_t[:],
                                     start=True, stop=True)
                    nc.vector.tensor_copy(out=h_state[:, g * H:(g + 1) * H], in_=ps[:])

                GBt = BBt = None
                for l in range(N_CONV):
                    wi_l = wi_t[:, l * 2 * H:(l + 1) * 2 * H]
                    wj_l = wj_t[:, l * 2 * H:(l + 1) * 2 * H]
                    wfs_l = wfs_t[:, l * 2 * H:(l + 1) * 2 * H]
                    hin_l = hin[l % 2]
                    hag_l = hag[l % 2]

                    # ---- phase A: (BN apply of l-1) + h table + hT ----
                    for g in range(G):
                        hsl = h_state[:, g * H:(g + 1) * H]
                        if l > 0:
                            t1 = sg.tile([P, H], f32, tag="t1")
                            nc.gpsimd.tensor_tensor(
                                out=t1[:], in0=agg_res[:, g * 2 * H:g * 2 * H + H],
                                in1=GBt[:], op=ALU.mult)
                            nc.gpsimd.tensor_tensor(out=t1[:], in0=t1[:],
                                                    in1=BBt[:], op=ALU.add)
                            nc.gpsimd.tensor_tensor(out=hsl, in0=hsl,
                                                    in1=t1[:], op=ALU.add)
                        htab = sg.tile([P, H], bf16, tag="htab")
                        nc.vector.tensor_copy(out=htab[:], in_=hsl)
                        nc.sync.dma_start(out=hin_l[g * P:(g + 1) * P, :], in_=htab[:])
                        psT = pst.tile([P, H], f32, space="PSUM", tag="tr")
                        nc.tensor.transpose(out=psT[:], in_=hsl, identity=id_f[:])
                        nc.vector.tensor_copy(out=hT_buf[:, g * H:(g + 1) * H],
                                              in_=psT[:])
                    nc.gpsimd.collective_compute(
                        "AllGather", ALU.bypass, replica_groups=[CORES],
                        ins=[hin_l[:].opt()], outs=[hag_l[:].opt()])

                    # ---- phase B: edge tiles ----
                    stat_ps = psst.tile([1, 2 * H], f32, space="PSUM", tag="stat")
                    for g in range(G):
                        eaTg = sg.tile([F_EDGE + 1, GSLOTS], bf16, tag="eaTg")
                        nc.sync.dma_start(
                            out=eaTg[:], in_=eaT_d[:, g * GSLOTS:(g + 1) * GSLOTS])
                        psAf = psaf.tile([P, 2 * H], f32, space="PSUM", tag="af")
                        nc.tensor.matmul(out=psAf[:],
                                         lhsT=hT_buf[:, g * H:(g + 1) * H],
                                         rhs=wi_l, start=True, stop=True)
                        afs = sg.tile([P, 2 * H], bf16, tag="afs")
                        nc.vector.tensor_copy(out=afs[:], in_=psAf[:])

                        agg_ps = psa.tile([P, H], f32, space="PSUM", tag="agg")
                        for t in range(KT):
                            k = g * KT + t
                            hjt = sb.tile([P, H], bf16, tag="hjt")
                            nc.gpsimd.indirect_dma_start(
                                out=hjt[:], out_offset=None, in_=hag_l[:],
                                in_offset=bass.IndirectOffsetOnAxis(
                                    ap=srcT_t[:, k:k + 1], axis=0))
                            o_t = sb.tile([P, P], bf16, tag="o_t")
                            nc.vector.tensor_scalar(
                                out=o_t[:], in0=iota_row[:],
                                scalar1=dstc_t[:, k:k + 1],
                                scalar2=None, op0=ALU.is_equal)
                            psO = pst.tile([P, P], bf16, space="PSUM", tag="tr")
                            nc.tensor.transpose(out=psO[:], in_=o_t[:],
                                                identity=id_bf[:])
                            oT = sb.tile([P, P], bf16, tag="oT")
                            nc.vector.tensor_copy(out=oT[:], in_=psO[:])
                            pre = psp.tile([P, 2 * H], f32, space="PSUM", tag="pp")
                            nc.tensor.matmul(out=pre[:],
                                             lhsT=eaTg[:, t * P:(t + 1) * P],
                                             rhs=wfs_l, start=True, stop=False)
                            nc.tensor.matmul(out=pre[:], lhsT=oT[:], rhs=afs[:],
                                             start=False, stop=False)
                            psHJ = pst.tile([P, H], bf16, space="PSUM", tag="tr")
                            nc.tensor.transpose(out=psHJ[:], in_=hjt[:],
                                                identity=id_bf[:])
                            hjT = sb.tile([P, H], bf16, tag="hjT")
                            nc.vector.tensor_copy(out=hjT[:], in_=psHJ[:])
                            nc.tensor.matmul(out=pre[:], lhsT=hjT[:], rhs=wj_l,
                                             start=False, stop=True)

                            ex2 = sb.tile([P, 2 * H], bf16, tag="ex2")
                            nc.scalar.activation(out=ex2[:], in_=pre[:], func=AF.Exp)
                            corr = sb.tile([P, H], bf16, tag="corr")
                            nc.scalar.activation(out=corr[:], in_=ex2[:, H:],
                                                 func=AF.Ln, bias=1.0)
                            den = sb.tile([P, H], bf16, tag="den")
                            nc.vector.tensor_scalar_add(den[:], ex2[:, :H], 1.0)
                            gate = sb.tile([P, H], bf16, tag="gate")
                            with nc.allow_low_precision("bf16 sigmoid gate"):
                                nc.vector.reciprocal(out=gate[:], in_=den[:])
                            msg = sb.tile([P, H], bf16, tag="msg")
                            nc.vector.tensor_tensor(out=msg[:], in0=corr[:],
                                                    in1=gate[:], op=ALU.mult)
                            nc.tensor.matmul(out=agg_ps[:], lhsT=o_t[:], rhs=msg[:],
                                             start=(t == 0), stop=(t == KT - 1))

                        agg_sl = agg_res[:, g * 2 * H:g * 2 * H + H]
                        sq_sl = agg_res[:, g * 2 * H + H:(g + 1) * 2 * H]
                        nc.vector.tensor_copy(out=agg_sl, in_=agg_ps[:])
                        nc.vector.tensor_tensor(out=sq_sl, in0=agg_sl,
                                                in1=agg_sl, op=ALU.mult)
                        nc.tensor.matmul(
                            out=stat_ps[:], lhsT=ones_col_bf[:],
                            rhs=agg_res[:, g * 2 * H:(g + 1) * 2 * H],
                            start=(g == 0), stop=(g == G - 1))

                    # ---- BN coefficients ----
                    stat_sb = sc.tile([1, 2 * H], f32, tag="stat_sb")
                    nc.vector.tensor_copy(out=stat_sb[:], in_=stat_ps[:])
                    nc.sync.dma_start(out=st_in[l][:], in_=stat_sb[:])
                    nc.gpsimd.collective_compute(
                        "AllReduce", ALU.add, replica_groups=[CORES],
                        ins=[st_in[l][:].opt()], outs=[st_out[l][:].opt()])
                    stg = sc.tile([1, 2 * H], f32, tag="stg")
                    nc.sync.dma_start(out=stg[:], in_=st_out[l][:])
                    mean = sc.tile([1, H], f32, tag="mean")
                    nc.vector.tensor_scalar_mul(mean[:], stg[:, :H], 1.0 / N_NODES)
                    msq = sc.tile([1, H], f32, tag="msq")
                    nc.vector.tensor_scalar_mul(msq[:], stg[:, H:], 1.0 / N_NODES)
                    m2 = sc.tile([1, H], f32, tag="m2")
                    nc.vector.tensor_tensor(out=m2[:], in0=mean[:], in1=mean[:], op=ALU.mult)
                    var = sc.tile([1, H], f32, tag="var")
                    nc.vector.tensor_tensor(out=var[:], in0=msq[:], in1=m2[:], op=ALU.subtract)
                    vareps = sc.tile([1, H], f32, tag="vareps")
                    nc.vector.tensor_scalar_add(vareps[:], var[:], BN_EPS)
                    sd = sc.tile([1, H], f32, tag="sd")
                    nc.scalar.activation(out=sd[:], in_=vareps[:], func=AF.Sqrt)
                    rstd = sc.tile([1, H], f32, tag="rstd")
                    nc.vector.reciprocal(out=rstd[:], in_=sd[:])
                    bngl = sc.tile([1, H], f32, tag="bngl")
                    nc.sync.dma_start(out=bngl[:], in_=bng_d[l:l + 1, :])
                    bnbl = sc.tile([1, H], f32, tag="bnbl")
                    nc.sync.dma_start(out=bnbl[:], in_=bnb_d[l:l + 1, :])
                    gco = sc.tile([1, H], f32, tag="gco")
                    nc.vector.tensor_tensor(out=gco[:], in0=rstd[:], in1=bngl[:], op=ALU.mult)
                    mg = sc.tile([1, H], f32, tag="mg")
                    nc.vector.tensor_tensor(out=mg[:], in0=mean[:], in1=gco[:], op=ALU.mult)
                    bco = sc.tile([1, H], f32, tag="bco")
                    nc.vector.tensor_tensor(out=bco[:], in0=bnbl[:], in1=mg[:], op=ALU.subtract)
                    psGB = pst.tile([P, H], f32, space="PSUM", tag="tr")
                    nc.tensor.matmul(out=psGB[:], lhsT=ones_row[:], rhs=gco[:], start=True, stop=True)
                    GBt = bnc.tile([P, H], f32, tag="GBt")
                    nc.vector.tensor_copy(out=GBt[:], in_=psGB[:])
                    psBB = pst.tile([P, H], f32, space="PSUM", tag="tr")
                    nc.tensor.matmul(out=psBB[:], lhsT=ones_row[:], rhs=bco[:], start=True, stop=True)
                    BBt = bnc.tile([P, H], f32, tag="BBt")
                    nc.vector.tensor_copy(out=BBt[:], in_=psBB[:])

                # ---- readout (BN of layer 2 fused) ----
                psW0 = psa.tile([P, H], f32, space="PSUM", tag="agg")
                psW1 = psa.tile([P, H], f32, space="PSUM", tag="agg")
                psW = [psW0, psW1]
                for g in range(G):
                    t1 = sg.tile([P, H], f32, tag="t1")
                    nc.gpsimd.tensor_tensor(out=t1[:],
                                            in0=agg_res[:, g * 2 * H:g * 2 * H + H],
                                            in1=GBt[:], op=ALU.mult)
                    nc.gpsimd.tensor_tensor(out=t1[:], in0=t1[:], in1=BBt[:], op=ALU.add)
                    nc.gpsimd.tensor_tensor(out=t1[:], in0=t1[:],
                                            in1=h_state[:, g * H:(g + 1) * H], op=ALU.add)
                    hb3 = sg.tile([P, H], bf16, tag="hb3")
                    nc.vector.tensor_copy(out=hb3[:], in_=t1[:])
                    pw = sg.tile([P, 256], bf16, tag="pw")
                    nc.sync.dma_start(out=pw[:], in_=poolw_d[:, g * 256:(g + 1) * 256])
                    for w in range(2):
                        nc.tensor.matmul(
                            out=psW[w][:], lhsT=pw[:, w * P:(w + 1) * P],
                            rhs=hb3[:], start=(g == 0), stop=(g == G - 1))
                zt = sb.tile([P, H], f32, tag="zt")
                nc.vector.memset(zt[:], 0.0)
                for i in range(GB_CAP // P):
                    nc.sync.dma_start(out=gbuf[i * P:(i + 1) * P, :], in_=zt[:])
                for w in range(2):
                    ws = sb.tile([P, H], f32, tag="ws")
                    nc.vector.tensor_copy(out=ws[:], in_=psW[w][:])
                    nc.gpsimd.indirect_dma_start(
                        out=gbuf[:],
                        out_offset=bass.IndirectOffsetOnAxis(ap=pids_t[:, w:w + 1], axis=0),
                        in_=ws[:], in_offset=None,
                        bounds_check=GB_CAP - 1, oob_is_err=False)
                nc.gpsimd.collective_compute(
                    "AllReduce", ALU.add, replica_groups=[CORES],
                    ins=[gbuf[:].opt()], outs=[gsum[:].opt()])

                psOW = pst.tile([P, H], f32, space="PSUM", tag="tr")
                nc.tensor.matmul(out=psOW[:], lhsT=ones_row[:], rhs=outw_t[:], start=True, stop=True)
                owb = sc.tile([P, H], f32, tag="owb")
                nc.vector.tensor_copy(out=owb[:], in_=psOW[:])

                for gb in range(GB_CAP // P):
                    gl = sb.tile([P, H], f32, tag="gl")
                    nc.sync.dma_start(out=gl[:], in_=gsum[gb * P:(gb + 1) * P, :])
                    gm = sb.tile([P, H], f32, tag="gm")
                    nc.vector.tensor_scalar(out=gm[:], in0=gl[:], scalar1=invc_t[:, gb:gb + 1],
                                            scalar2=None, op0=ALU.mult)
                    psT2 = pst.tile([P, H], f32, space="PSUM", tag="tr")
                    nc.tensor.transpose(out=psT2[:], in_=gm[:], identity=id_f[:])
                    gT2 = sb.tile([P, H], f32, tag="gT2")
                    nc.vector.tensor_copy(out=gT2[:], in_=psT2[:])
                    psF = psp.tile([P, H], f32, space="PSUM", tag="pp")
                    nc.tensor.matmul(out=psF[:], lhsT=gT2[:], rhs=fcw_t[:], start=True, stop=False)
                    nc.tensor.matmul(out=psF[:], lhsT=ones_row[:], rhs=fcb_t[:], start=False, stop=True)
                    ex = sb.tile([P, H], f32, tag="ex")
                    nc.scalar.activation(out=ex[:], in_=psF[:], func=AF.Exp)
                    sp = sb.tile([P, H], f32, tag="sp")
                    nc.scalar.activation(out=sp[:], in_=ex[:], func=AF.Ln, bias=1.0)
                    mu = sb.tile([P, H], f32, tag="mu")
                    nc.vector.tensor_tensor(out=mu[:], in0=sp[:], in1=owb[:], op=ALU.mult)
                    red = sb.tile([P, 1], f32, tag="red")
                    nc.vector.tensor_reduce(out=red[:], in_=mu[:], axis=mybir.AxisListType.X, op=ALU.add)
                    redb = sb.tile([P, 1], f32, tag="redb")
                    nc.vector.tensor_scalar(out=redb[:], in0=red[:], scalar1=outb_t[:, :1],
                                            scalar2=None, op0=ALU.add)
                    nc.sync.dma_start(out=out_d[gb * P:(gb + 1) * P, :], in_=redb[:])

    nc.compile()
    return nc


def get_program(G, reps=1):
    key = (G, reps)
    if key not in _CACHE:
        _CACHE[key] = build_program(G, reps)
    return _CACHE[key]


def kernel(x, edge_attr, emb_W, emb_b, lin_f_W, lin_f_b, lin_s_W, lin_s_b,
           bn_gamma, bn_beta, fc_W, fc_b, out_W, out_b, edge_index, batch):
    params = dict(emb_W=emb_W, emb_b=emb_b, lin_f_W=lin_f_W, lin_f_b=lin_f_b,
                  lin_s_W=lin_s_W, lin_s_b=lin_s_b, bn_gamma=bn_gamma,
                  bn_beta=bn_beta, fc_W=fc_W, fc_b=fc_b, out_W=out_W, out_b=out_b)
    in_maps, G = pack_host(x, edge_attr, edge_index, batch, params)
    nc = get_program(G)
    res = run_bass_kernel_spmd(nc, in_maps, list(range(NCORES)))
    out = res.results[0]["out"]
    return np.asarray(out, dtype=np.float32).reshape(GB_CAP)[:N_GRAPHS]
